# revision 3
# baseline (speedup 1.0000x reference)
"""Tied-attention (MSA-style) kernel for 8 TRN2 NeuronCores.

Problem: x (32,1024,256) f32; q/kv projections; tied attention over the
r=32 MSA-row dim (logits summed over r); softmax; out-projection + bias.

Sharding: tensor-parallel by heads (8 heads -> 1 head per core). Each core
computes q/k/v for its head from the full (host-pre-transposed, bf16-cast)
x, accumulates its head's tied logits S^T = sum_r k_r q_r^T entirely
locally (no collective), softmaxes along the PSUM partition axis via a
ones-matmul, applies attention, then AllToAll redistributes per-head
outputs into per-core row shards (4 rows of r each). Each core finishes
with the full output projection for its rows; the host concatenates and
un-transposes the shards.

Compute dtype: bf16 operands, f32 PSUM accumulation, f32 softmax.
"""
import numpy as np
import ml_dtypes

import concourse.bacc as bacc
import concourse.mybir as mybir
import concourse.tile as tile
from concourse.bass_utils import run_bass_kernel_spmd

dt = mybir.dt
BF16 = ml_dtypes.bfloat16

H, D, R, N, DIM = 8, 64, 32, 1024, 256
INNER = H * D          # 512
ROWS = R * N           # 32768
NPAIR = R // 2         # 16
NCORES = 8
RL = R // NCORES       # 4 rows of r per core after AllToAll
SCALE = (D ** -0.5) * (R ** -0.5)

_NC_CACHE = None


def _build():
    nc = bacc.Bacc("TRN2", target_bir_lowering=False, debug=False, num_devices=NCORES)

    xt = nc.dram_tensor("xt", [DIM, ROWS], dt.bfloat16, kind="ExternalInput")
    wq = nc.dram_tensor("wq", [DIM, D], dt.bfloat16, kind="ExternalInput")
    wk = nc.dram_tensor("wk", [DIM, D], dt.bfloat16, kind="ExternalInput")
    wv = nc.dram_tensor("wv", [DIM, D], dt.bfloat16, kind="ExternalInput")
    wout = nc.dram_tensor("wout", [INNER, DIM], dt.bfloat16, kind="ExternalInput")
    bias = nc.dram_tensor("bias", [1, DIM], dt.bfloat16, kind="ExternalInput")
    yt = nc.dram_tensor("yt", [DIM, RL * N], dt.float32, kind="ExternalOutput")

    with tile.TileContext(nc) as tc:
        with (
            tc.tile_pool(name="dram", bufs=1, space="DRAM") as dram,
            tc.tile_pool(name="persist", bufs=1) as per,
            tc.tile_pool(name="xc", bufs=4) as xcp,
            tc.tile_pool(name="stage", bufs=4) as stg,
            tc.tile_pool(name="gio", bufs=3) as gio,
        ):
            # A2A buffers: shard d = out^T rows r = 4d..4d+3 (bf16)
            a2a_in = dram.tile([NCORES, RL, D, N], dt.bfloat16)
            a2a_out = dram.tile([NCORES, RL, D, N], dt.bfloat16)

            # persistent SBUF tensors
            wq_sb = per.tile([128, 2, D], dt.bfloat16, tag="wq")
            wk_sb = per.tile([128, 2, D], dt.bfloat16, tag="wk")
            wv_sb = per.tile([128, 2, D], dt.bfloat16, tag="wv")
            wout_sb = per.tile([128, 4, DIM], dt.bfloat16, tag="wout")
            bias_sb = per.tile([1, DIM], dt.bfloat16, tag="bias")
            ones_col = per.tile([128, 1], dt.bfloat16, tag="ones_col")
            ones_row = per.tile([1, 512], dt.bfloat16, tag="ones_row")
            den_sb = per.tile([1, N], dt.float32, tag="den")
            recip_sb = per.tile([1, N], dt.float32, tag="recip")
            bc_sb = per.tile([128, N], dt.float32, tag="bc")
            # per-pair persistent: q^T/k^T [(r-parity, d), n], v row-major
            qts = [per.tile([128, N], dt.bfloat16, tag=f"qt{p}", name=f"qt{p}") for p in range(NPAIR)]
            kts = [per.tile([128, N], dt.bfloat16, tag=f"kt{p}", name=f"kt{p}") for p in range(NPAIR)]
            vs = [per.tile([128, 8, 128], dt.bfloat16, tag=f"v{p}", name=f"v{p}") for p in range(NPAIR)]
            # P^T tiles per (jc, ih): [j-in-chunk, i-half]
            pts = [[per.tile([128, 512], dt.bfloat16, tag=f"pt{jc}_{ih}", name=f"pt{jc}_{ih}")
                    for ih in range(2)] for jc in range(8)]

            nc.sync.dma_start(wq_sb[:], wq.ap().rearrange("(a p) m -> p a m", p=128))
            nc.sync.dma_start(wk_sb[:], wk.ap().rearrange("(a p) m -> p a m", p=128))
            nc.sync.dma_start(wv_sb[:], wv.ap().rearrange("(a p) m -> p a m", p=128))
            nc.sync.dma_start(wout_sb[:], wout.ap().rearrange("(a p) m -> p a m", p=128))
            nc.sync.dma_start(bias_sb[:], bias[:])
            nc.vector.memset(ones_col[:], 1.0)
            nc.vector.memset(ones_row[:], 1.0)

            # ---- Phase 1: projections q^T,k^T (parity layout) + v (row major) ----
            with tc.tile_pool(name="ps_proj", bufs=4, space="PSUM") as psp:
                for p in range(NPAIR):
                    xc = [xcp.tile([128, 2 * N], dt.bfloat16, tag="xc", name=f"xc{p}_{i}") for i in range(2)]
                    for kt in range(2):
                        nc.sync.dma_start(
                            xc[kt][:],
                            xt[kt * 128:(kt + 1) * 128, 2 * p * N:(2 * p + 2) * N])
                    for nh in range(2):
                        sl_r0 = slice(nh * 512, nh * 512 + 512)
                        sl_r1 = slice(N + nh * 512, N + nh * 512 + 512)
                        pq = psp.tile([128, 512], dt.float32, tag="proj")
                        pk = psp.tile([128, 512], dt.float32, tag="proj")
                        pv = psp.tile([128, 512], dt.float32, tag="proj")
                        for w_sb, ps in ((wq_sb, pq), (wk_sb, pk), (wv_sb, pv)):
                            for kt in range(2):
                                nc.tensor.matmul(ps[0:64, :], w_sb[:, kt, :],
                                                 xc[kt][:, sl_r0],
                                                 start=(kt == 0), stop=(kt == 1),
                                                 tile_position=(0, 0))
                                nc.tensor.matmul(ps[64:128, :], w_sb[:, kt, :],
                                                 xc[kt][:, sl_r1],
                                                 start=(kt == 0), stop=(kt == 1),
                                                 tile_position=(0, 64))
                        sl_n = slice(nh * 512, nh * 512 + 512)
                        nc.vector.tensor_copy(qts[p][:, sl_n], pq[:])
                        nc.vector.tensor_copy(kts[p][:, sl_n], pk[:])
                        vstage = stg.tile([128, 512], dt.bfloat16, tag="vstage")
                        nc.vector.tensor_copy(vstage[:], pv[:])
                        # transpose v^T (parity,d)xn -> n x (parity,d)
                        for sub in range(4):
                            jc = nh * 4 + sub
                            nc.sync.dma_start_transpose(
                                vs[p][:, jc, :], vstage[:, sub * 128:(sub + 1) * 128])

            # ---- Phase 2: S^T = sum_r k_r q_r^T (per j-chunk), softmax ----
            with (
                tc.tile_pool(name="ps_s", bufs=2, space="PSUM") as pss,
                tc.tile_pool(name="ps_den", bufs=1, space="PSUM") as psd,
            ):
                pden = psd.tile([1, N], dt.float32, tag="den")
                for ih in range(2):
                    sl_i = slice(ih * 512, ih * 512 + 512)
                    for jc in range(8):
                        ps = pss.tile([128, 512], dt.float32, tag="s")
                        for p in range(NPAIR):
                            nc.tensor.matmul(ps[:], kts[p][:, jc * 128:(jc + 1) * 128],
                                             qts[p][:, sl_i],
                                             start=(p == 0), stop=(p == NPAIR - 1))
                        nc.scalar.activation(pts[jc][ih][:], ps[:],
                                             mybir.ActivationFunctionType.Exp,
                                             scale=SCALE)
                        nc.tensor.matmul(pden[0:1, sl_i], ones_col[:], pts[jc][ih][:],
                                         start=(jc == 0), stop=(jc == 7))
                nc.vector.tensor_copy(den_sb[:], pden[:])
            nc.vector.reciprocal(recip_sb[:], den_sb[:])
            nc.gpsimd.partition_broadcast(bc_sb[:], recip_sb[:])

            # ---- Phase 3: out^T_r = v_r^T P^T, normalized on evacuation ----
            with tc.tile_pool(name="ps_av", bufs=4, space="PSUM") as psa:
                for p in range(NPAIR):
                    r0, r1 = 2 * p, 2 * p + 1
                    for ih in range(2):
                        sl_i = slice(ih * 512, ih * 512 + 512)
                        po = psa.tile([128, 512], dt.float32, tag="av")
                        for jc in range(8):
                            nc.tensor.matmul(po[0:64, :], vs[p][:, jc, 0:64],
                                             pts[jc][ih][:],
                                             start=(jc == 0), stop=(jc == 7),
                                             tile_position=(0, 0))
                            nc.tensor.matmul(po[64:128, :], vs[p][:, jc, 64:128],
                                             pts[jc][ih][:],
                                             start=(jc == 0), stop=(jc == 7),
                                             tile_position=(0, 64))
                        osb = stg.tile([128, 512], dt.bfloat16, tag="osb")
                        nc.vector.tensor_mul(osb[:], po[:], bc_sb[:, sl_i])
                        nc.sync.dma_start(
                            a2a_in[r0 // RL, r0 % RL, :, ih * 512:ih * 512 + 512],
                            osb[0:64, :])
                        nc.sync.dma_start(
                            a2a_in[r1 // RL, r1 % RL, :, ih * 512:ih * 512 + 512],
                            osb[64:128, :])

            # ---- Phase 4: AllToAll: head-sharded -> row-sharded ----
            nc.gpsimd.collective_compute(
                "AllToAll",
                mybir.AluOpType.bypass,
                replica_groups=[list(range(NCORES))],
                ins=[a2a_in.opt()],
                outs=[a2a_out.opt()],
            )

            # ---- Phase 5: y^T = Wout^T out + bias for own 4 r-rows ----
            with tc.tile_pool(name="ps_y", bufs=4, space="PSUM") as psy:
                for rl in range(RL):
                    g = gio.tile([128, 4, N], dt.bfloat16, tag="g")
                    for kt in range(4):
                        nc.sync.dma_start(g[:, kt, :],
                                          a2a_out[2 * kt:2 * kt + 2, rl, :, :])
                    for m in range(2):
                        sl_m = slice(m * 128, m * 128 + 128)
                        for nh in range(2):
                            sl_n = slice(nh * 512, nh * 512 + 512)
                            py = psy.tile([128, 512], dt.float32, tag="y")
                            for kt in range(4):
                                nc.tensor.matmul(py[:], wout_sb[:, kt, sl_m],
                                                 g[:, kt, sl_n],
                                                 start=(kt == 0), stop=False)
                            nc.tensor.matmul(py[:], bias_sb[:, sl_m], ones_row[:],
                                             start=False, stop=True)
                            ysb = gio.tile([128, 512], dt.float32, tag="ysb")
                            nc.vector.tensor_copy(ysb[:], py[:])
                            nc.sync.dma_start(
                                yt[sl_m, rl * N + nh * 512: rl * N + nh * 512 + 512],
                                ysb[:])
    nc.finalize()
    return nc


def kernel(x, Wq, Wkv, Wout, bout, tie_attn_dim):
    global _NC_CACHE
    assert int(tie_attn_dim) == R
    x = np.asarray(x, dtype=np.float32)
    xt = np.ascontiguousarray(x.reshape(ROWS, DIM).T).astype(BF16)
    wout_b = np.asarray(Wout, np.float32).astype(BF16)
    bias_b = np.asarray(bout, np.float32).reshape(1, DIM).astype(BF16)
    Wq = np.asarray(Wq, np.float32)
    Wkv = np.asarray(Wkv, np.float32)

    in_maps = []
    for c in range(NCORES):
        sl = slice(c * D, (c + 1) * D)
        in_maps.append({
            "xt": xt,
            "wq": np.ascontiguousarray(Wq[:, sl]).astype(BF16),
            "wk": np.ascontiguousarray(Wkv[:, sl]).astype(BF16),
            "wv": np.ascontiguousarray(Wkv[:, INNER + c * D:INNER + (c + 1) * D]).astype(BF16),
            "wout": wout_b,
            "bias": bias_b,
        })

    if _NC_CACHE is None:
        _NC_CACHE = _build()
    res = run_bass_kernel_spmd(_NC_CACHE, in_maps, core_ids=list(range(NCORES)))

    y = np.empty((R, N, DIM), dtype=np.float32)
    for c in range(NCORES):
        ytc = res.results[c]["yt"]  # (DIM, RL*N)
        y[c * RL:(c + 1) * RL] = ytc.reshape(DIM, RL, N).transpose(1, 2, 0)
    return y


# revision 5
# speedup vs baseline: 1.0952x; 1.0952x over previous
"""Tied-attention (MSA-style) kernel for 8 TRN2 NeuronCores.

Problem: x (32,1024,256) f32; q/kv projections; tied attention over the
r=32 MSA-row dim (logits summed over r); softmax; out-projection + bias.

Sharding: tensor-parallel by heads (8 heads -> 1 head per core). Each core
computes q/k/v for its head from the full (host-pre-transposed, bf16-cast)
x, accumulates its head's tied logits S^T = sum_r k_r q_r^T entirely
locally (no collective), softmaxes along the PSUM partition axis via a
ones-matmul, applies attention, then AllToAll redistributes per-head
outputs into per-core row shards (4 rows of r each). Each core finishes
with the full output projection for its rows; the host concatenates and
un-transposes the shards.

Compute dtype: bf16 operands, f32 PSUM accumulation, f32 softmax.
"""
import numpy as np
import ml_dtypes

import concourse.bacc as bacc
import concourse.mybir as mybir
import concourse.tile as tile
from concourse.bass_utils import run_bass_kernel_spmd

dt = mybir.dt
BF16 = ml_dtypes.bfloat16

H, D, R, N, DIM = 8, 64, 32, 1024, 256
INNER = H * D          # 512
ROWS = R * N           # 32768
NPAIR = R // 2         # 16
NCORES = 8
RL = R // NCORES       # 4 rows of r per core after AllToAll
SCALE = (D ** -0.5) * (R ** -0.5)

_NC_CACHE = None


def _build():
    nc = bacc.Bacc("TRN2", target_bir_lowering=False, debug=False, num_devices=NCORES)

    xt = nc.dram_tensor("xt", [DIM, ROWS], dt.bfloat16, kind="ExternalInput")
    wq = nc.dram_tensor("wq", [DIM, D], dt.bfloat16, kind="ExternalInput")
    wk = nc.dram_tensor("wk", [DIM, D], dt.bfloat16, kind="ExternalInput")
    wv = nc.dram_tensor("wv", [DIM, D], dt.bfloat16, kind="ExternalInput")
    wout = nc.dram_tensor("wout", [INNER, DIM], dt.bfloat16, kind="ExternalInput")
    bias = nc.dram_tensor("bias", [1, DIM], dt.bfloat16, kind="ExternalInput")
    yt = nc.dram_tensor("yt", [DIM, RL * N], dt.float32, kind="ExternalOutput")

    with tile.TileContext(nc) as tc:
        with (
            tc.tile_pool(name="dram", bufs=1, space="DRAM") as dram,
            tc.tile_pool(name="persist", bufs=1) as per,
            tc.tile_pool(name="xc", bufs=4) as xcp,
            tc.tile_pool(name="stage", bufs=4) as stg,
            tc.tile_pool(name="gio", bufs=3) as gio,
        ):
            # A2A buffers: chunk h shard d = out^T rows {4d+2h, 4d+2h+1} (bf16)
            a2a_ins = [dram.tile([NCORES, 2, D, N], dt.bfloat16, name=f"a2ai{h}")
                       for h in range(2)]
            a2a_outs = [dram.tile([NCORES, 2, D, N], dt.bfloat16, name=f"a2ao{h}")
                        for h in range(2)]

            # persistent SBUF tensors
            wq_sb = per.tile([128, 2, D], dt.bfloat16, tag="wq")
            wk_sb = per.tile([128, 2, D], dt.bfloat16, tag="wk")
            wv_sb = per.tile([128, 2, D], dt.bfloat16, tag="wv")
            wout_sb = per.tile([128, 4, DIM], dt.bfloat16, tag="wout")
            bias_sb = per.tile([1, DIM], dt.bfloat16, tag="bias")
            ones_col = per.tile([128, 1], dt.bfloat16, tag="ones_col")
            ones_row = per.tile([1, N], dt.bfloat16, tag="ones_row")
            den_sb = per.tile([1, N], dt.float32, tag="den")
            recip_sb = per.tile([1, N], dt.float32, tag="recip")
            bc_sb = per.tile([128, N], dt.float32, tag="bc")
            # per-pair persistent: q^T/k^T [(r-parity, d), n], v row-major
            qts = [per.tile([128, N], dt.bfloat16, tag=f"qt{p}", name=f"qt{p}")
                   for p in range(NPAIR)]
            kts = [per.tile([128, N], dt.bfloat16, tag=f"kt{p}", name=f"kt{p}")
                   for p in range(NPAIR)]
            vs = [per.tile([128, 8, 128], dt.bfloat16, tag=f"v{p}", name=f"v{p}")
                  for p in range(NPAIR)]
            # P^T tiles per jc: [j-in-chunk, i]
            pts = [per.tile([128, N], dt.bfloat16, tag=f"pt{jc}", name=f"pt{jc}")
                   for jc in range(8)]

            nc.sync.dma_start(wq_sb[:], wq.ap().rearrange("(a p) m -> p a m", p=128))
            nc.sync.dma_start(wk_sb[:], wk.ap().rearrange("(a p) m -> p a m", p=128))
            nc.sync.dma_start(wv_sb[:], wv.ap().rearrange("(a p) m -> p a m", p=128))
            nc.sync.dma_start(wout_sb[:], wout.ap().rearrange("(a p) m -> p a m", p=128))
            nc.sync.dma_start(bias_sb[:], bias[:])
            nc.vector.memset(ones_col[:], 1.0)
            nc.vector.memset(ones_row[:], 1.0)

            # ---- Phase 1: projections q^T,k^T (parity layout) + v (row major) ----
            with tc.tile_pool(name="ps_proj", bufs=4, space="PSUM") as psp:
                for p in range(NPAIR):
                    xc = [xcp.tile([128, 2 * N], dt.bfloat16, tag="xc", name=f"xc{p}_{i}")
                          for i in range(2)]
                    for kt in range(2):
                        nc.sync.dma_start(
                            xc[kt][:],
                            xt[kt * 128:(kt + 1) * 128, 2 * p * N:(2 * p + 2) * N])
                    pq = psp.tile([128, N], dt.float32, tag="proj", name=f"pq{p}")
                    pk = psp.tile([128, N], dt.float32, tag="proj", name=f"pk{p}")
                    pv = psp.tile([128, N], dt.float32, tag="proj", name=f"pv{p}")
                    for w_sb, ps in ((wq_sb, pq), (wk_sb, pk), (wv_sb, pv)):
                        for kt in range(2):
                            for col, base in ((0, 0), (64, N)):
                                for nh in range(2):
                                    sl = slice(base + nh * 512, base + nh * 512 + 512)
                                    nc.tensor.matmul(
                                        ps[col:col + 64, nh * 512:nh * 512 + 512],
                                        w_sb[:, kt, :], xc[kt][:, sl],
                                        start=(kt == 0), stop=(kt == 1),
                                        tile_position=(0, col))
                    nc.vector.tensor_copy(qts[p][:], pq[:])
                    nc.vector.tensor_copy(kts[p][:], pk[:])
                    vstage = stg.tile([128, N], dt.bfloat16, tag="vstage",
                                      name=f"vst{p}")
                    nc.vector.tensor_copy(vstage[:], pv[:])
                    # transpose v^T (parity,d)xn -> n x (parity,d); scalar DGE queue
                    for jc in range(8):
                        nc.scalar.dma_start_transpose(
                            vs[p][:, jc, :], vstage[:, jc * 128:(jc + 1) * 128])

            # ---- Phase 2: S^T = sum_r k_r q_r^T (per j-chunk), softmax ----
            with (
                tc.tile_pool(name="ps_s", bufs=2, space="PSUM") as pss,
                tc.tile_pool(name="ps_den", bufs=1, space="PSUM") as psd,
            ):
                pden = psd.tile([1, N], dt.float32, tag="den")
                for jc in range(8):
                    ps = pss.tile([128, N], dt.float32, tag="s", name=f"s{jc}")
                    for p in range(NPAIR):
                        for ih in range(2):
                            nc.tensor.matmul(
                                ps[:, ih * 512:ih * 512 + 512],
                                kts[p][:, jc * 128:(jc + 1) * 128],
                                qts[p][:, ih * 512:ih * 512 + 512],
                                start=(p == 0), stop=(p == NPAIR - 1))
                    nc.scalar.activation(pts[jc][:], ps[:],
                                         mybir.ActivationFunctionType.Exp,
                                         scale=SCALE)
                    for ih in range(2):
                        nc.tensor.matmul(pden[:, ih * 512:ih * 512 + 512],
                                         ones_col[:],
                                         pts[jc][:, ih * 512:ih * 512 + 512],
                                         start=(jc == 0), stop=(jc == 7))
                nc.vector.tensor_copy(den_sb[:], pden[:])
            nc.vector.reciprocal(recip_sb[:], den_sb[:])
            nc.gpsimd.partition_broadcast(bc_sb[:], recip_sb[:])

            # ---- Phase 3 + 4: attention-weighted values; two overlapped A2As.
            # Even pairs p feed A2A chunk 0 (rows 4d,4d+1), odd pairs chunk 1
            # (rows 4d+2,4d+3); chunk 0's collective overlaps odd-pair compute.
            with tc.tile_pool(name="ps_av", bufs=3, space="PSUM") as psa:
                for half in range(2):
                    for p in range(half, NPAIR, 2):
                        r0, r1 = 2 * p, 2 * p + 1
                        po = psa.tile([128, N], dt.float32, tag="av", name=f"av{p}")
                        for jc in range(8):
                            for col, dsl in ((0, slice(0, 64)), (64, slice(64, 128))):
                                for ih in range(2):
                                    nc.tensor.matmul(
                                        po[dsl, ih * 512:ih * 512 + 512],
                                        vs[p][:, jc, col:col + 64],
                                        pts[jc][:, ih * 512:ih * 512 + 512],
                                        start=(jc == 0), stop=(jc == 7),
                                        tile_position=(0, col))
                        osb = stg.tile([128, N], dt.bfloat16, tag="osb",
                                       name=f"osb{p}")
                        nc.vector.tensor_mul(osb[:], po[:], bc_sb[:])
                        dest = p // 2
                        nc.sync.dma_start(a2a_ins[half][dest, 0, :, :], osb[0:64, :])
                        nc.sync.dma_start(a2a_ins[half][dest, 1, :, :], osb[64:128, :])
                    nc.gpsimd.collective_compute(
                        "AllToAll",
                        mybir.AluOpType.bypass,
                        replica_groups=[list(range(NCORES))],
                        ins=[a2a_ins[half].opt()],
                        outs=[a2a_outs[half].opt()],
                    )

            # ---- Phase 5: y^T = Wout^T out + bias for own 4 r-rows ----
            with tc.tile_pool(name="ps_y", bufs=4, space="PSUM") as psy:
                for rl in range(RL):
                    half, sub = rl // 2, rl % 2
                    g = gio.tile([128, 4, N], dt.bfloat16, tag="g", name=f"g{rl}")
                    for kt in range(4):
                        nc.sync.dma_start(g[:, kt, :],
                                          a2a_outs[half][2 * kt:2 * kt + 2, sub, :, :])
                    for m in range(2):
                        sl_m = slice(m * 128, m * 128 + 128)
                        py = psy.tile([128, N], dt.float32, tag="y",
                                      name=f"py{rl}_{m}")
                        for kt in range(4):
                            for nh in range(2):
                                nc.tensor.matmul(py[:, nh * 512:nh * 512 + 512],
                                                 wout_sb[:, kt, sl_m],
                                                 g[:, kt, nh * 512:nh * 512 + 512],
                                                 start=(kt == 0), stop=False)
                        for nh in range(2):
                            nc.tensor.matmul(py[:, nh * 512:nh * 512 + 512],
                                             bias_sb[:, sl_m],
                                             ones_row[:, nh * 512:nh * 512 + 512],
                                             start=False, stop=True)
                        ysb = gio.tile([128, N], dt.float32, tag="ysb",
                                       name=f"ysb{rl}_{m}")
                        nc.vector.tensor_copy(ysb[:], py[:])
                        nc.sync.dma_start(yt[sl_m, rl * N:(rl + 1) * N], ysb[:])
    nc.finalize()
    return nc


def kernel(x, Wq, Wkv, Wout, bout, tie_attn_dim):
    global _NC_CACHE
    assert int(tie_attn_dim) == R
    x = np.asarray(x, dtype=np.float32)
    xt = np.ascontiguousarray(x.reshape(ROWS, DIM).T).astype(BF16)
    wout_b = np.asarray(Wout, np.float32).astype(BF16)
    bias_b = np.asarray(bout, np.float32).reshape(1, DIM).astype(BF16)
    Wq = np.asarray(Wq, np.float32)
    Wkv = np.asarray(Wkv, np.float32)

    in_maps = []
    for c in range(NCORES):
        sl = slice(c * D, (c + 1) * D)
        in_maps.append({
            "xt": xt,
            "wq": np.ascontiguousarray(Wq[:, sl]).astype(BF16),
            "wk": np.ascontiguousarray(Wkv[:, sl]).astype(BF16),
            "wv": np.ascontiguousarray(Wkv[:, INNER + c * D:INNER + (c + 1) * D]).astype(BF16),
            "wout": wout_b,
            "bias": bias_b,
        })

    if _NC_CACHE is None:
        _NC_CACHE = _build()
    res = run_bass_kernel_spmd(_NC_CACHE, in_maps, core_ids=list(range(NCORES)))

    y = np.empty((R, N, DIM), dtype=np.float32)
    for c in range(NCORES):
        ytc = res.results[c]["yt"]  # (DIM, RL*N)
        y[c * RL:(c + 1) * RL] = ytc.reshape(DIM, RL, N).transpose(1, 2, 0)
    return y


# revision 8
# speedup vs baseline: 1.3386x; 1.2222x over previous
"""Tied-attention (MSA-style) kernel for 8 TRN2 NeuronCores.

Problem: x (32,1024,256) f32; q/kv projections; tied attention over the
r=32 MSA-row dim (logits summed over r); softmax; out-projection + bias.

Sharding: tensor-parallel by heads (8 heads -> 1 head per core). Each core
computes q/k/v for its head from the full (host-pre-transposed, bf16-cast)
x, accumulates its head's tied logits S^T = sum_r k_r q_r^T entirely
locally (no collective), softmaxes along the PSUM partition axis via a
ones-matmul, applies attention, then two AllToAlls redistribute per-head
outputs into per-core row shards (4 rows of r each); the first one
overlaps the second half of the attention compute. Each core finishes
with the full output projection for its rows; the host concatenates and
un-transposes the shards.

v is produced head-transposed and flipped to row-major with PE-mode
transposes (DMA transpose serializes the whole DMA subsystem via
xbar_mode transitions - measured 12us/pair stalls - so it is avoided).

Compute dtype: bf16 operands, f32 PSUM accumulation, f32 softmax.
"""
import numpy as np
import ml_dtypes

import concourse.bacc as bacc
import concourse.mybir as mybir
import concourse.tile as tile
from concourse.bass_utils import run_bass_kernel_spmd

dt = mybir.dt
BF16 = ml_dtypes.bfloat16

H, D, R, N, DIM = 8, 64, 32, 1024, 256
INNER = H * D          # 512
ROWS = R * N           # 32768
NPAIR = R // 2         # 16
NCORES = 8
RL = R // NCORES       # 4 rows of r per core after AllToAll
SCALE = (D ** -0.5) * (R ** -0.5)

_NC_CACHE = None


def _build():
    nc = bacc.Bacc("TRN2", target_bir_lowering=False, debug=False, num_devices=NCORES)

    xt = nc.dram_tensor("xt", [DIM, ROWS], dt.bfloat16, kind="ExternalInput")
    wq = nc.dram_tensor("wq", [DIM, D], dt.bfloat16, kind="ExternalInput")
    wk = nc.dram_tensor("wk", [DIM, D], dt.bfloat16, kind="ExternalInput")
    wv = nc.dram_tensor("wv", [DIM, D], dt.bfloat16, kind="ExternalInput")
    wout = nc.dram_tensor("wout", [INNER, DIM], dt.bfloat16, kind="ExternalInput")
    bias = nc.dram_tensor("bias", [1, DIM], dt.bfloat16, kind="ExternalInput")
    ident = nc.dram_tensor("ident", [128, 128], dt.bfloat16, kind="ExternalInput")
    yt = nc.dram_tensor("yt", [DIM, RL * N], dt.float32, kind="ExternalOutput")

    with tile.TileContext(nc) as tc:
        with (
            tc.tile_pool(name="dram", bufs=1, space="DRAM") as dram,
            tc.tile_pool(name="persist", bufs=1) as per,
            tc.tile_pool(name="xc", bufs=4) as xcp,
            tc.tile_pool(name="stage", bufs=4) as stg,
            tc.tile_pool(name="gio", bufs=2) as gio,
        ):
            # A2A buffers: chunk h shard d = out^T rows {4d+2h, 4d+2h+1} (bf16)
            a2a_ins = [dram.tile([NCORES, 2, D, N], dt.bfloat16, name=f"a2ai{h}")
                       for h in range(2)]
            a2a_outs = [dram.tile([NCORES, 2, D, N], dt.bfloat16, name=f"a2ao{h}")
                        for h in range(2)]

            # persistent SBUF tensors
            wq_sb = per.tile([128, 2, D], dt.bfloat16, tag="wq")
            wk_sb = per.tile([128, 2, D], dt.bfloat16, tag="wk")
            wv_sb = per.tile([128, 2, D], dt.bfloat16, tag="wv")
            wout_sb = per.tile([128, 4, DIM], dt.bfloat16, tag="wout")
            bias_sb = per.tile([1, DIM], dt.bfloat16, tag="bias")
            ident_sb = per.tile([128, 128], dt.bfloat16, tag="ident")
            ones_col = per.tile([128, 1], dt.bfloat16, tag="ones_col")
            ones_row = per.tile([1, N], dt.bfloat16, tag="ones_row")
            den_sb = per.tile([1, N], dt.float32, tag="den")
            bcf_sb = per.tile([128, N], dt.float32, tag="bcf")
            bc_sb = per.tile([128, N], dt.bfloat16, tag="bc")
            # per-pair persistent: q^T/k^T [(r-parity, d), n], v row-major
            qts = [per.tile([128, N], dt.bfloat16, tag=f"qt{p}", name=f"qt{p}")
                   for p in range(NPAIR)]
            kts = [per.tile([128, N], dt.bfloat16, tag=f"kt{p}", name=f"kt{p}")
                   for p in range(NPAIR)]
            vs = [per.tile([128, 8, 128], dt.bfloat16, tag=f"v{p}", name=f"v{p}")
                  for p in range(NPAIR)]
            # P^T tiles per jc: [j-in-chunk, i]
            pts = [per.tile([128, N], dt.bfloat16, tag=f"pt{jc}", name=f"pt{jc}")
                   for jc in range(8)]

            nc.sync.dma_start(wq_sb[:], wq.ap().rearrange("(a p) m -> p a m", p=128))
            nc.sync.dma_start(wk_sb[:], wk.ap().rearrange("(a p) m -> p a m", p=128))
            nc.sync.dma_start(wv_sb[:], wv.ap().rearrange("(a p) m -> p a m", p=128))
            nc.sync.dma_start(wout_sb[:], wout.ap().rearrange("(a p) m -> p a m", p=128))
            nc.sync.dma_start(bias_sb[:], bias[:])
            nc.sync.dma_start(ident_sb[:], ident[:])
            nc.vector.memset(ones_col[:], 1.0)
            nc.vector.memset(ones_row[:], 1.0)

            # ---- Phase 1: projections q^T,k^T (parity layout) + v (row major) ----
            with (
                tc.tile_pool(name="ps_proj", bufs=3, space="PSUM") as psp,
                tc.tile_pool(name="ps_vtr", bufs=2, space="PSUM") as psv,
            ):
                for p in range(NPAIR):
                    xc = [xcp.tile([128, 2 * N], dt.bfloat16, tag="xc", name=f"xc{p}_{i}")
                          for i in range(2)]
                    for kt in range(2):
                        nc.sync.dma_start(
                            xc[kt][:],
                            xt[kt * 128:(kt + 1) * 128, 2 * p * N:(2 * p + 2) * N])
                    pq = psp.tile([128, N], dt.float32, tag="proj", name=f"pq{p}")
                    pk = psp.tile([128, N], dt.float32, tag="proj", name=f"pk{p}")
                    pv = psp.tile([128, N], dt.float32, tag="proj", name=f"pv{p}")
                    for w_sb, ps in ((wq_sb, pq), (wk_sb, pk), (wv_sb, pv)):
                        for kt in range(2):
                            for col, base in ((0, 0), (64, N)):
                                for nh in range(2):
                                    sl = slice(base + nh * 512, base + nh * 512 + 512)
                                    nc.tensor.matmul(
                                        ps[col:col + 64, nh * 512:nh * 512 + 512],
                                        w_sb[:, kt, :], xc[kt][:, sl],
                                        start=(kt == 0), stop=(kt == 1),
                                        tile_position=(0, col))
                    nc.scalar.activation(qts[p][:], pq[:],
                                         mybir.ActivationFunctionType.Copy)
                    nc.vector.tensor_copy(kts[p][:], pk[:])
                    vstage = stg.tile([128, N], dt.bfloat16, tag="vstage",
                                      name=f"vst{p}")
                    nc.vector.tensor_copy(vstage[:], pv[:])
                    # PE-transpose v^T (parity,d)xn -> n x (parity,d)
                    for jc in range(8):
                        pt_ps = psv.tile([128, 128], dt.bfloat16, tag="vtr",
                                         name=f"vtr{p}_{jc}")
                        nc.tensor.transpose(pt_ps[:],
                                            vstage[:, jc * 128:(jc + 1) * 128],
                                            ident_sb[:])
                        if jc % 2 == 0:
                            nc.vector.tensor_copy(vs[p][:, jc, :], pt_ps[:])
                        else:
                            nc.scalar.activation(vs[p][:, jc, :], pt_ps[:],
                                                 mybir.ActivationFunctionType.Copy)

            # ---- Phase 2: S^T = sum_r k_r q_r^T (per j-chunk), softmax ----
            with (
                tc.tile_pool(name="ps_s", bufs=2, space="PSUM") as pss,
                tc.tile_pool(name="ps_den", bufs=1, space="PSUM") as psd,
            ):
                pden = psd.tile([1, N], dt.float32, tag="den")
                for jc in range(8):
                    ps = pss.tile([128, N], dt.float32, tag="s", name=f"s{jc}")
                    for p in range(NPAIR):
                        for ih in range(2):
                            nc.tensor.matmul(
                                ps[:, ih * 512:ih * 512 + 512],
                                kts[p][:, jc * 128:(jc + 1) * 128],
                                qts[p][:, ih * 512:ih * 512 + 512],
                                start=(p == 0), stop=(p == NPAIR - 1))
                    nc.scalar.activation(pts[jc][:], ps[:],
                                         mybir.ActivationFunctionType.Exp,
                                         scale=SCALE)
                    for ih in range(2):
                        nc.tensor.matmul(pden[:, ih * 512:ih * 512 + 512],
                                         ones_col[:],
                                         pts[jc][:, ih * 512:ih * 512 + 512],
                                         start=(jc == 0), stop=(jc == 7))
                nc.vector.tensor_copy(den_sb[:], pden[:])
            # broadcast first, then full-width reciprocal (fast on 128 lanes)
            nc.gpsimd.partition_broadcast(bcf_sb[:], den_sb[:])
            nc.vector.reciprocal(bcf_sb[:], bcf_sb[:])
            nc.vector.tensor_copy(bc_sb[:], bcf_sb[:])
            # normalize P in place (bf16 4x DVE) so evacuations are pure copies
            for jc in range(8):
                nc.vector.tensor_mul(pts[jc][:], pts[jc][:], bc_sb[:])

            # ---- Phase 3 + 4: attention-weighted values; two overlapped A2As.
            # Even pairs p feed A2A chunk 0 (rows 4d,4d+1), odd pairs chunk 1
            # (rows 4d+2,4d+3); chunk 0's collective overlaps odd-pair compute.
            with tc.tile_pool(name="ps_av", bufs=3, space="PSUM") as psa:
                for half in range(2):
                    for p in range(half, NPAIR, 2):
                        po = psa.tile([128, N], dt.float32, tag="av", name=f"av{p}")
                        for jc in range(8):
                            for ih in range(2):
                                nc.tensor.matmul(
                                    po[:, ih * 512:ih * 512 + 512],
                                    vs[p][:, jc, :],
                                    pts[jc][:, ih * 512:ih * 512 + 512],
                                    start=(jc == 0), stop=(jc == 7))
                        osb = stg.tile([128, N], dt.bfloat16, tag="osb",
                                       name=f"osb{p}")
                        if (p // 2) % 2 == 0:
                            nc.vector.tensor_copy(osb[:], po[:])
                        else:
                            nc.scalar.activation(osb[:], po[:],
                                                 mybir.ActivationFunctionType.Copy)
                        dest = p // 2
                        nc.sync.dma_start(a2a_ins[half][dest, 0, :, :], osb[0:64, :])
                        nc.sync.dma_start(a2a_ins[half][dest, 1, :, :], osb[64:128, :])
                    nc.gpsimd.collective_compute(
                        "AllToAll",
                        mybir.AluOpType.bypass,
                        replica_groups=[list(range(NCORES))],
                        ins=[a2a_ins[half].opt()],
                        outs=[a2a_outs[half].opt()],
                    )

            # ---- Phase 5: y^T = Wout^T out + bias for own 4 r-rows ----
            with tc.tile_pool(name="ps_y", bufs=4, space="PSUM") as psy:
                for rl in range(RL):
                    half, sub = rl // 2, rl % 2
                    g = gio.tile([128, 4, N], dt.bfloat16, tag="g", name=f"g{rl}")
                    for kt in range(4):
                        nc.sync.dma_start(g[:, kt, :],
                                          a2a_outs[half][2 * kt:2 * kt + 2, sub, :, :])
                    for m in range(2):
                        sl_m = slice(m * 128, m * 128 + 128)
                        py = psy.tile([128, N], dt.float32, tag="y",
                                      name=f"py{rl}_{m}")
                        for kt in range(4):
                            for nh in range(2):
                                nc.tensor.matmul(py[:, nh * 512:nh * 512 + 512],
                                                 wout_sb[:, kt, sl_m],
                                                 g[:, kt, nh * 512:nh * 512 + 512],
                                                 start=(kt == 0), stop=False)
                        for nh in range(2):
                            nc.tensor.matmul(py[:, nh * 512:nh * 512 + 512],
                                             bias_sb[:, sl_m],
                                             ones_row[:, nh * 512:nh * 512 + 512],
                                             start=False, stop=True)
                        ysb = gio.tile([128, N], dt.float32, tag="ysb",
                                       name=f"ysb{rl}_{m}")
                        if m == 0:
                            nc.vector.tensor_copy(ysb[:], py[:])
                        else:
                            nc.scalar.activation(ysb[:], py[:],
                                                 mybir.ActivationFunctionType.Copy)
                        nc.sync.dma_start(yt[sl_m, rl * N:(rl + 1) * N], ysb[:])
    nc.finalize()
    return nc


def kernel(x, Wq, Wkv, Wout, bout, tie_attn_dim):
    global _NC_CACHE
    assert int(tie_attn_dim) == R
    x = np.asarray(x, dtype=np.float32)
    xt = np.ascontiguousarray(x.reshape(ROWS, DIM).T).astype(BF16)
    wout_b = np.asarray(Wout, np.float32).astype(BF16)
    bias_b = np.asarray(bout, np.float32).reshape(1, DIM).astype(BF16)
    ident = np.eye(128, dtype=BF16)
    Wq = np.asarray(Wq, np.float32)
    Wkv = np.asarray(Wkv, np.float32)

    in_maps = []
    for c in range(NCORES):
        sl = slice(c * D, (c + 1) * D)
        in_maps.append({
            "xt": xt,
            "wq": np.ascontiguousarray(Wq[:, sl]).astype(BF16),
            "wk": np.ascontiguousarray(Wkv[:, sl]).astype(BF16),
            "wv": np.ascontiguousarray(Wkv[:, INNER + c * D:INNER + (c + 1) * D]).astype(BF16),
            "wout": wout_b,
            "bias": bias_b,
            "ident": ident,
        })

    if _NC_CACHE is None:
        _NC_CACHE = _build()
    res = run_bass_kernel_spmd(_NC_CACHE, in_maps, core_ids=list(range(NCORES)))

    y = np.empty((R, N, DIM), dtype=np.float32)
    for c in range(NCORES):
        ytc = res.results[c]["yt"]  # (DIM, RL*N)
        y[c * RL:(c + 1) * RL] = ytc.reshape(DIM, RL, N).transpose(1, 2, 0)
    return y


# revision 9
# speedup vs baseline: 1.9066x; 1.4244x over previous
"""Tied-attention (MSA-style) kernel for 8 TRN2 NeuronCores.

Problem: x (32,1024,256) f32; q/kv projections; tied attention over the
r=32 MSA-row dim (logits summed over r); softmax; out-projection + bias.

Sharding: tensor-parallel by heads (8 heads -> 1 head per core). Each core
computes q/k/v for its head from the full (host-pre-transposed, bf16-cast)
x, accumulates its head's tied logits S^T = sum_r k_r q_r^T entirely
locally (no collective), softmaxes along the PSUM partition axis via a
ones-matmul, applies attention, then two AllToAlls redistribute per-head
outputs into per-core row shards (4 rows of r each); the first one
overlaps the second half of the attention compute. Each core finishes
with the full output projection for its rows; the host concatenates and
un-transposes the shards.

v is produced head-transposed and flipped to row-major with PE-mode
transposes (DMA transpose serializes the whole DMA subsystem via
xbar_mode transitions - measured 12us/pair stalls - so it is avoided).

Compute dtype: bf16 operands, f32 PSUM accumulation, f32 softmax.
"""
import numpy as np
import ml_dtypes

import concourse.bacc as bacc
import concourse.mybir as mybir
import concourse.tile as tile
from concourse.bass_utils import run_bass_kernel_spmd

dt = mybir.dt
BF16 = ml_dtypes.bfloat16

H, D, R, N, DIM = 8, 64, 32, 1024, 256
INNER = H * D          # 512
ROWS = R * N           # 32768
NPAIR = R // 2         # 16
NCORES = 8
RL = R // NCORES       # 4 rows of r per core after AllToAll
SCALE = (D ** -0.5) * (R ** -0.5)

_NC_CACHE = None


def _build():
    nc = bacc.Bacc("TRN2", target_bir_lowering=False, debug=False, num_devices=NCORES)

    xt = nc.dram_tensor("xt", [DIM, ROWS], dt.bfloat16, kind="ExternalInput")
    wq = nc.dram_tensor("wq", [DIM, D], dt.bfloat16, kind="ExternalInput")
    wk = nc.dram_tensor("wk", [DIM, D], dt.bfloat16, kind="ExternalInput")
    wv = nc.dram_tensor("wv", [DIM, D], dt.bfloat16, kind="ExternalInput")
    wout = nc.dram_tensor("wout", [INNER, DIM], dt.bfloat16, kind="ExternalInput")
    bias = nc.dram_tensor("bias", [1, DIM], dt.bfloat16, kind="ExternalInput")
    ident = nc.dram_tensor("ident", [128, 128], dt.bfloat16, kind="ExternalInput")
    yt = nc.dram_tensor("yt", [DIM, RL * N], dt.float32, kind="ExternalOutput")

    with tile.TileContext(nc) as tc:
        with (
            tc.tile_pool(name="dram", bufs=1, space="DRAM") as dram,
            tc.tile_pool(name="persist", bufs=1) as per,
            tc.tile_pool(name="xc", bufs=4) as xcp,
            tc.tile_pool(name="stage", bufs=4) as stg,
            tc.tile_pool(name="gio", bufs=2) as gio,
        ):
            # A2A buffers: chunk h shard d = out^T rows {4d+2h, 4d+2h+1} (bf16)
            a2a_ins = [dram.tile([NCORES, 2, D, N], dt.bfloat16, name=f"a2ai{h}")
                       for h in range(2)]
            a2a_outs = [dram.tile([NCORES, 2, D, N], dt.bfloat16, name=f"a2ao{h}")
                        for h in range(2)]

            # persistent SBUF tensors
            wq_sb = per.tile([128, 2, D], dt.bfloat16, tag="wq")
            wk_sb = per.tile([128, 2, D], dt.bfloat16, tag="wk")
            wv_sb = per.tile([128, 2, D], dt.bfloat16, tag="wv")
            wout_sb = per.tile([128, 4, DIM], dt.bfloat16, tag="wout")
            bias_sb = per.tile([1, DIM], dt.bfloat16, tag="bias")
            ident_sb = per.tile([128, 128], dt.bfloat16, tag="ident")
            ones_col = per.tile([128, 1], dt.bfloat16, tag="ones_col")
            ones_row = per.tile([1, N], dt.bfloat16, tag="ones_row")
            den_sb = per.tile([1, N], dt.float32, tag="den")
            bcf_sb = per.tile([128, N], dt.float32, tag="bcf")
            bc_sb = per.tile([128, N], dt.bfloat16, tag="bc")
            # per-pair persistent: q^T/k^T [(r-parity, d), n], v row-major
            qts = [per.tile([128, N], dt.bfloat16, tag=f"qt{p}", name=f"qt{p}")
                   for p in range(NPAIR)]
            kts = [per.tile([128, N], dt.bfloat16, tag=f"kt{p}", name=f"kt{p}")
                   for p in range(NPAIR)]
            vs = [per.tile([128, 8, 128], dt.bfloat16, tag=f"v{p}", name=f"v{p}")
                  for p in range(NPAIR)]
            # P^T tiles per jc: [j-in-chunk, i]
            pts = [per.tile([128, N], dt.bfloat16, tag=f"pt{jc}", name=f"pt{jc}")
                   for jc in range(8)]

            nc.sync.dma_start(wq_sb[:], wq.ap().rearrange("(a p) m -> p a m", p=128))
            nc.sync.dma_start(wk_sb[:], wk.ap().rearrange("(a p) m -> p a m", p=128))
            nc.sync.dma_start(wv_sb[:], wv.ap().rearrange("(a p) m -> p a m", p=128))
            nc.sync.dma_start(wout_sb[:], wout.ap().rearrange("(a p) m -> p a m", p=128))
            nc.sync.dma_start(bias_sb[:], bias[:])
            nc.sync.dma_start(ident_sb[:], ident[:])
            nc.vector.memset(ones_col[:], 1.0)
            nc.vector.memset(ones_row[:], 1.0)

            # warm-up collective: absorbs cross-core start skew and ncfw cold
            # init while phase 1 computes; CC engine only.
            warm_in = dram.tile([1, 64], dt.float32, name="warm_in")
            warm_out = dram.tile([NCORES, 64], dt.float32, name="warm_out")
            nc.sync.dma_start(warm_in[:], den_sb[0:1, 0:64])
            nc.gpsimd.collective_compute(
                "AllGather",
                mybir.AluOpType.bypass,
                replica_groups=[list(range(NCORES))],
                ins=[warm_in.opt()],
                outs=[warm_out.opt()],
            )

            # ---- Phase 1: projections q^T,k^T (parity layout) + v (row major) ----
            with (
                tc.tile_pool(name="ps_proj", bufs=3, space="PSUM") as psp,
                tc.tile_pool(name="ps_vtr", bufs=2, space="PSUM") as psv,
            ):
                for p in range(NPAIR):
                    xc = [xcp.tile([128, 2 * N], dt.bfloat16, tag="xc", name=f"xc{p}_{i}")
                          for i in range(2)]
                    for kt in range(2):
                        nc.sync.dma_start(
                            xc[kt][:],
                            xt[kt * 128:(kt + 1) * 128, 2 * p * N:(2 * p + 2) * N])
                    pq = psp.tile([128, N], dt.float32, tag="proj", name=f"pq{p}")
                    pk = psp.tile([128, N], dt.float32, tag="proj", name=f"pk{p}")
                    pv = psp.tile([128, N], dt.float32, tag="proj", name=f"pv{p}")
                    for w_sb, ps in ((wq_sb, pq), (wk_sb, pk), (wv_sb, pv)):
                        for kt in range(2):
                            for col, base in ((0, 0), (64, N)):
                                for nh in range(2):
                                    sl = slice(base + nh * 512, base + nh * 512 + 512)
                                    nc.tensor.matmul(
                                        ps[col:col + 64, nh * 512:nh * 512 + 512],
                                        w_sb[:, kt, :], xc[kt][:, sl],
                                        start=(kt == 0), stop=(kt == 1),
                                        tile_position=(0, col))
                    nc.scalar.activation(qts[p][:], pq[:],
                                         mybir.ActivationFunctionType.Copy)
                    nc.vector.tensor_copy(kts[p][:], pk[:])
                    vstage = stg.tile([128, N], dt.bfloat16, tag="vstage",
                                      name=f"vst{p}")
                    nc.vector.tensor_copy(vstage[:], pv[:])
                    # PE-transpose v^T (parity,d)xn -> n x (parity,d)
                    for jc in range(8):
                        pt_ps = psv.tile([128, 128], dt.bfloat16, tag="vtr",
                                         name=f"vtr{p}_{jc}")
                        nc.tensor.transpose(pt_ps[:],
                                            vstage[:, jc * 128:(jc + 1) * 128],
                                            ident_sb[:])
                        if jc % 2 == 0:
                            nc.vector.tensor_copy(vs[p][:, jc, :], pt_ps[:])
                        else:
                            nc.scalar.activation(vs[p][:, jc, :], pt_ps[:],
                                                 mybir.ActivationFunctionType.Copy)

            # ---- Phase 2: S^T = sum_r k_r q_r^T (per j-chunk), softmax ----
            with (
                tc.tile_pool(name="ps_s", bufs=2, space="PSUM") as pss,
                tc.tile_pool(name="ps_den", bufs=1, space="PSUM") as psd,
            ):
                pden = psd.tile([1, N], dt.float32, tag="den")
                for jc in range(8):
                    ps = pss.tile([128, N], dt.float32, tag="s", name=f"s{jc}")
                    for p in range(NPAIR):
                        for ih in range(2):
                            nc.tensor.matmul(
                                ps[:, ih * 512:ih * 512 + 512],
                                kts[p][:, jc * 128:(jc + 1) * 128],
                                qts[p][:, ih * 512:ih * 512 + 512],
                                start=(p == 0), stop=(p == NPAIR - 1))
                    nc.scalar.activation(pts[jc][:], ps[:],
                                         mybir.ActivationFunctionType.Exp,
                                         scale=SCALE)
                    for ih in range(2):
                        nc.tensor.matmul(pden[:, ih * 512:ih * 512 + 512],
                                         ones_col[:],
                                         pts[jc][:, ih * 512:ih * 512 + 512],
                                         start=(jc == 0), stop=(jc == 7))
                nc.vector.tensor_copy(den_sb[:], pden[:])
            # broadcast first, then full-width reciprocal (fast on 128 lanes)
            nc.gpsimd.partition_broadcast(bcf_sb[:], den_sb[:])
            nc.vector.reciprocal(bcf_sb[:], bcf_sb[:])
            nc.vector.tensor_copy(bc_sb[:], bcf_sb[:])
            # normalize P in place (bf16 4x DVE) so evacuations are pure copies
            for jc in range(8):
                nc.vector.tensor_mul(pts[jc][:], pts[jc][:], bc_sb[:])

            # ---- Phase 3 + 4: attention-weighted values; two overlapped A2As.
            # Even pairs p feed A2A chunk 0 (rows 4d,4d+1), odd pairs chunk 1
            # (rows 4d+2,4d+3); chunk 0's collective overlaps odd-pair compute.
            with tc.tile_pool(name="ps_av", bufs=3, space="PSUM") as psa:
                for half in range(2):
                    for p in range(half, NPAIR, 2):
                        po = psa.tile([128, N], dt.float32, tag="av", name=f"av{p}")
                        for jc in range(8):
                            for ih in range(2):
                                nc.tensor.matmul(
                                    po[:, ih * 512:ih * 512 + 512],
                                    vs[p][:, jc, :],
                                    pts[jc][:, ih * 512:ih * 512 + 512],
                                    start=(jc == 0), stop=(jc == 7))
                        osb = stg.tile([128, N], dt.bfloat16, tag="osb",
                                       name=f"osb{p}")
                        if (p // 2) % 2 == 0:
                            nc.vector.tensor_copy(osb[:], po[:])
                        else:
                            nc.scalar.activation(osb[:], po[:],
                                                 mybir.ActivationFunctionType.Copy)
                        dest = p // 2
                        nc.sync.dma_start(a2a_ins[half][dest, 0, :, :], osb[0:64, :])
                        nc.sync.dma_start(a2a_ins[half][dest, 1, :, :], osb[64:128, :])
                    nc.gpsimd.collective_compute(
                        "AllToAll",
                        mybir.AluOpType.bypass,
                        replica_groups=[list(range(NCORES))],
                        ins=[a2a_ins[half].opt()],
                        outs=[a2a_outs[half].opt()],
                    )

            # ---- Phase 5: y^T = Wout^T out + bias for own 4 r-rows ----
            with tc.tile_pool(name="ps_y", bufs=4, space="PSUM") as psy:
                for rl in range(RL):
                    half, sub = rl // 2, rl % 2
                    g = gio.tile([128, 4, N], dt.bfloat16, tag="g", name=f"g{rl}")
                    for kt in range(4):
                        nc.sync.dma_start(g[0:64, kt, :],
                                          a2a_outs[half][2 * kt, sub, :, :])
                        nc.sync.dma_start(g[64:128, kt, :],
                                          a2a_outs[half][2 * kt + 1, sub, :, :])
                    for m in range(2):
                        sl_m = slice(m * 128, m * 128 + 128)
                        py = psy.tile([128, N], dt.float32, tag="y",
                                      name=f"py{rl}_{m}")
                        for kt in range(4):
                            for nh in range(2):
                                nc.tensor.matmul(py[:, nh * 512:nh * 512 + 512],
                                                 wout_sb[:, kt, sl_m],
                                                 g[:, kt, nh * 512:nh * 512 + 512],
                                                 start=(kt == 0), stop=False)
                        for nh in range(2):
                            nc.tensor.matmul(py[:, nh * 512:nh * 512 + 512],
                                             bias_sb[:, sl_m],
                                             ones_row[:, nh * 512:nh * 512 + 512],
                                             start=False, stop=True)
                        ysb = gio.tile([128, N], dt.float32, tag="ysb",
                                       name=f"ysb{rl}_{m}")
                        if m == 0:
                            nc.vector.tensor_copy(ysb[:], py[:])
                        else:
                            nc.scalar.activation(ysb[:], py[:],
                                                 mybir.ActivationFunctionType.Copy)
                        nc.gpsimd.dma_start(yt[sl_m, rl * N:(rl + 1) * N], ysb[:])
    nc.finalize()
    return nc


def kernel(x, Wq, Wkv, Wout, bout, tie_attn_dim):
    global _NC_CACHE
    assert int(tie_attn_dim) == R
    x = np.asarray(x, dtype=np.float32)
    xt = np.ascontiguousarray(x.reshape(ROWS, DIM).T).astype(BF16)
    wout_b = np.asarray(Wout, np.float32).astype(BF16)
    bias_b = np.asarray(bout, np.float32).reshape(1, DIM).astype(BF16)
    ident = np.eye(128, dtype=BF16)
    Wq = np.asarray(Wq, np.float32)
    Wkv = np.asarray(Wkv, np.float32)

    in_maps = []
    for c in range(NCORES):
        sl = slice(c * D, (c + 1) * D)
        in_maps.append({
            "xt": xt,
            "wq": np.ascontiguousarray(Wq[:, sl]).astype(BF16),
            "wk": np.ascontiguousarray(Wkv[:, sl]).astype(BF16),
            "wv": np.ascontiguousarray(Wkv[:, INNER + c * D:INNER + (c + 1) * D]).astype(BF16),
            "wout": wout_b,
            "bias": bias_b,
            "ident": ident,
        })

    if _NC_CACHE is None:
        _NC_CACHE = _build()
    res = run_bass_kernel_spmd(_NC_CACHE, in_maps, core_ids=list(range(NCORES)))

    y = np.empty((R, N, DIM), dtype=np.float32)
    for c in range(NCORES):
        ytc = res.results[c]["yt"]  # (DIM, RL*N)
        y[c * RL:(c + 1) * RL] = ytc.reshape(DIM, RL, N).transpose(1, 2, 0)
    return y
